# revision 27
# baseline (speedup 1.0000x reference)
"""DGF layer kernel for Trainium2 (Bass/Tile), data-parallel over batch.

Reference computation (per sample, N=1024, D=256, H=8 heads):
    sq[n]   = sum_d x[n,d]^2
    dist2   = sq[n] + sq[m] - 2*x@x.T               (clamped >= 0)
    adj     = mean_h exp(-dist2 / (2*exp(ls_h)^2 + 1e-6))
    out     = LN(elu(adj @ x @ W.T + b) + x) * gamma + beta

Kernel strategy (one sample per NeuronCore, 8 cores):
    - A short burst of dummy matmuls at kernel start warms the PE HAM clock
      (1.2 -> 2.4 GHz); HWDGE input DMAs have ~3-5us issue-to-complete
      latency, so the warmup burst exactly covers the wait for x^T.
    - Gram matmuls in bf16; ACT computes E = exp(2a*G - a*sq[n]) with a
      per-partition bias; a DVE bf16 multiply by the host-precomputed row
      R[m] = (cnt/H)*exp(-a*sq[m]) completes adj.  E overflows to inf on
      the diagonal and R underflows to 0, giving NaN there; a gpsimd
      affine_select overwrites the diagonal with the exact value 1.0.
    - Everything downstream of the adjacency runs single-bf16 (y = x@W.T,
      agg = adj@y).  The 2e-2 rel-err budget dwarfs bf16 rounding.
    - elu(z) + x = max(z,0) + min(exp(z),1) + (x - 1), split across engines:
      gpsimd computes t = min(exp,1) + (x-1), DVE computes v = max(z,0) + t
      with accum_out = per-row sum(v) for the LayerNorm mean; an ACT Square
      pass emits accum_out = sum(v^2) for the variance.  rstd via batched
      quake-seed Newton rsqrt (1 iteration) on DVE; the normalize
      (v*rstd + (-mean*rstd)) is spread over gpsimd and DVE.
    - Output stored as bf16 (upcast to f32 on host) in three batched DMAs.
"""

from contextlib import ExitStack

import numpy as np

B, N, D, H = 8, 1024, 256, 8
LN_EPS = 1e-5
P = 128
NT = N // P  # 8 row/col blocks
DC = D // P  # 2 contraction chunks

_PROGRAM_CACHE = {}


def _build_program(n_alpha, two_alphas, need_b, need_gamma, need_beta):
    import concourse.bass as bass
    import concourse.tile as tile
    from concourse import bacc, mybir

    f32 = mybir.dt.float32
    bf16 = mybir.dt.bfloat16
    i32 = mybir.dt.int32
    AF = mybir.ActivationFunctionType
    OP = mybir.AluOpType

    nc = bacc.Bacc("TRN2", target_bir_lowering=False, debug=False, enable_asserts=False)

    # Big inputs are pre-blocked on the host so each SBUF partition's data is
    # one contiguous DRAM row (4KB DMA packets instead of 512B-2KB ones).
    xhi_d = nc.dram_tensor("xhi", [P, DC * N], bf16, kind="ExternalInput").ap()
    xm1_d = nc.dram_tensor("xm1", [P, NT * D], bf16, kind="ExternalInput").ap()
    whi_d = nc.dram_tensor("whi", [D, D], bf16, kind="ExternalInput").ap()
    nasq_d = nc.dram_tensor("nasq", [P, n_alpha * NT], f32, kind="ExternalInput").ap()
    rbf_d = nc.dram_tensor("rbf", [n_alpha, N], bf16, kind="ExternalInput").ap()
    brow_d = grow_d = berow_d = None
    if need_b:
        brow_d = nc.dram_tensor("brow", [D], f32, kind="ExternalInput").ap()
    if need_gamma:
        grow_d = nc.dram_tensor("grow", [D], f32, kind="ExternalInput").ap()
    if need_beta:
        berow_d = nc.dram_tensor("berow", [D], f32, kind="ExternalInput").ap()
    out_d = nc.dram_tensor("out", [N, D], bf16, kind="ExternalOutput").ap()

    def bcast_ap(src):
        return bass.AP(tensor=src.tensor, offset=src.offset,
                       ap=[[0, P]] + list(src.ap))

    with tile.TileContext(nc) as tc, ExitStack() as ctx:
        singles = ctx.enter_context(tc.tile_pool(name="singles", bufs=1))
        work = ctx.enter_context(tc.tile_pool(name="work", bufs=4))
        stats = ctx.enter_context(tc.tile_pool(name="stats", bufs=4))

        # ---- persistent SBUF tensors ----
        xhi_sb = singles.tile([P, DC, N], bf16)
        xm1_sb = singles.tile([P, NT, D], bf16)
        whi_sb = singles.tile([P, DC, D], bf16)
        yhi_sb = singles.tile([P, NT, D], bf16)
        adj_sb = singles.tile([P, NT, N], bf16)
        v_sb = singles.tile([P, NT, D], bf16)
        out_sb = singles.tile([P, NT, D], bf16)
        nasq_sb = singles.tile([P, n_alpha, NT], f32)
        rbf_sb = singles.tile([P, n_alpha, N], bf16)
        sum_sb = singles.tile([P, NT], f32)
        sumsq_sb = singles.tile([P, NT], f32)
        mean_sb = singles.tile([P, NT], f32)
        rstd_sb = singles.tile([P, NT], f32)
        nmr_sb = singles.tile([P, NT], f32)
        magic_sb = singles.tile([P, NT], i32)

        # ---- input DMAs first; xhi alone on the sync ring so nothing
        # interleaves ahead of its completion ----
        nc.sync.dma_start(out=xhi_sb[:], in_=xhi_d.rearrange("p (c n) -> p c n", c=DC))
        nc.sync.dma_start(out=nasq_sb[:],
                          in_=nasq_d.rearrange("p (u t) -> p u t", u=n_alpha))
        for u in range(n_alpha):
            nc.scalar.dma_start(out=rbf_sb[:, u, :], in_=bcast_ap(rbf_d[u]))
        nc.scalar.dma_start(out=whi_sb[:], in_=whi_d.rearrange("(c p) e -> p c e", p=P))
        nc.gpsimd.dma_start(out=xm1_sb[:], in_=xm1_d.rearrange("p (t d) -> p t d", t=NT))
        b_bc = g_bc = be_bc = None
        if need_b:
            b_bc = singles.tile([P, D], f32)
            nc.scalar.dma_start(out=b_bc[:], in_=bcast_ap(brow_d))
        if need_gamma:
            g_bc = singles.tile([P, D], f32)
            nc.scalar.dma_start(out=g_bc[:], in_=bcast_ap(grow_d))
        if need_beta:
            be_bc = singles.tile([P, D], f32)
            nc.scalar.dma_start(out=be_bc[:], in_=bcast_ap(berow_d))

        warm_sb = singles.tile([P, 512], bf16)
        nc.vector.memset(warm_sb[:], 0.5)
        nc.vector.memset(magic_sb[:], 0x5F3759DF)

        # ---- PE warmup while input DMAs land (HAM 1.2 -> 2.4 GHz).
        # HWDGE issue-to-complete latency for x^T is ~4us; 8 cold dummy
        # matmuls (~3.4us) cover the wait and flip the HAM to full clock. ----
        warm_pool = tc.tile_pool(name="warm_psum", bufs=1, space="PSUM")
        wp = warm_pool.__enter__()
        pwarm = wp.tile([P, 512], f32)
        for _ in range(8):
            nc.tensor.matmul(pwarm[:], warm_sb[:, 0:P], warm_sb[:],
                             start=True, stop=True)
        warm_pool.__exit__(None, None, None)

        # ---- adjacency rows; y matmuls slipped into PE slack mid-stream ----
        g_pool = tc.tile_pool(name="g_psum", bufs=3, space="PSUM")
        gp = g_pool.__enter__()
        y_pool = tc.tile_pool(name="y_psum", bufs=2, space="PSUM")
        yp = y_pool.__enter__()

        def y_group(grp):
            py = yp.tile([P, 2, D], f32)
            for jj in range(2):
                j = 2 * grp + jj
                for c in range(DC):
                    nc.tensor.matmul(
                        py[:, jj, :],
                        xhi_sb[:, c, j * P:(j + 1) * P],
                        whi_sb[:, c, :],
                        start=(c == 0),
                        stop=(c == DC - 1),
                    )
            nc.vector.tensor_copy(yhi_sb[:, 2 * grp:2 * grp + 2, :], py[:])

        for a in range(NT):
            pg = gp.tile([P, N], f32)
            for c in range(DC):
                for h in range(2):
                    sl = slice(h * 512, (h + 1) * 512)
                    nc.tensor.matmul(
                        pg[:, sl],
                        xhi_sb[:, c, a * P:(a + 1) * P],
                        xhi_sb[:, c, sl],
                        start=(c == 0),
                        stop=(c == DC - 1),
                    )
            for u in range(n_alpha):
                et = work.tile([P, N], bf16, tag="et")
                nc.scalar.activation(
                    et[:], pg[:], AF.Exp,
                    bias=nasq_sb[:, u, a:a + 1], scale=two_alphas[u],
                )
                if u == 0:
                    nc.vector.tensor_tensor(
                        adj_sb[:, a, :], et[:], rbf_sb[:, 0, :], OP.mult
                    )
                else:
                    tmp = work.tile([P, N], bf16, tag="tmpu")
                    nc.vector.tensor_tensor(tmp[:], et[:], rbf_sb[:, u, :], OP.mult)
                    nc.vector.tensor_tensor(
                        adj_sb[:, a, :], adj_sb[:, a, :], tmp[:], OP.add
                    )
            # dist2(n,n) == 0 -> diagonal is exactly sum_u cnt_u/H = 1; this
            # also overwrites the inf*0 NaNs that E*R produces there.
            nc.gpsimd.affine_select(
                out=adj_sb[:, a, a * P:(a + 1) * P],
                in_=adj_sb[:, a, a * P:(a + 1) * P],
                compare_op=OP.not_equal,
                fill=1.0, base=0, channel_multiplier=1, pattern=[[-1, P]],
            )
            # y = x @ W.T rides the exp-paced PE slack mid-stream
            if a == 3:
                y_group(0)
                y_group(1)
            if a == 5:
                y_group(2)
                y_group(3)
        y_pool.__exit__(None, None, None)
        g_pool.__exit__(None, None, None)

        # ---- agg = adj @ y, elu, +x, fused stats, LN ----
        a_pool = tc.tile_pool(name="a_psum", bufs=8, space="PSUM")
        ap_ = a_pool.__enter__()

        def stats_batch(bi):
            # mean = sum/D; var = sumsq/D - mean^2 (eps dropped: var >> 1e-5);
            # rstd via quake-seed + one Newton step on DVE (ACT Ln/Rsqrt live
            # in a different activation-table set -- switching costs 1.3us).
            bs = slice(bi * 4, bi * 4 + 4)
            # mean (for nmr) is off the critical chain; msq' = sum^2/D^2 in a
            # single fused op keeps the dependent chain at 8 hops.
            nc.vector.tensor_scalar(
                mean_sb[:, bs], sum_sb[:, bs], 1.0 / D, None, OP.mult
            )
            msq = stats.tile([P, 4], f32, tag="msq")
            nc.vector.scalar_tensor_tensor(
                msq[:], sum_sb[:, bs], 1.0 / (D * D), sum_sb[:, bs], OP.mult, OP.mult
            )
            wv = stats.tile([P, 4], f32, tag="wv")
            nc.vector.scalar_tensor_tensor(
                wv[:], sumsq_sb[:, bs], 1.0 / D, msq[:], OP.mult, OP.subtract
            )
            sh = stats.tile([P, 4], i32, tag="sh")
            nc.vector.tensor_scalar(
                sh[:], wv[:].bitcast(i32), 1, None, OP.arith_shift_right
            )
            nc.vector.tensor_tensor(
                rstd_sb[:, bs].bitcast(i32), magic_sb[:, bs], sh[:], OP.subtract
            )
            na = stats.tile([P, 4], f32, tag="na")
            nb = stats.tile([P, 4], f32, tag="nb")
            nc.vector.tensor_tensor(na[:], rstd_sb[:, bs], rstd_sb[:, bs], OP.mult)
            nc.vector.scalar_tensor_tensor(nb[:], na[:], -0.5, wv[:], OP.mult, OP.mult)
            nc.vector.scalar_tensor_tensor(
                rstd_sb[:, bs], nb[:], 1.5, rstd_sb[:, bs], OP.add, OP.mult
            )
            nc.vector.scalar_tensor_tensor(
                nmr_sb[:, bs], mean_sb[:, bs], -1.0, rstd_sb[:, bs], OP.mult, OP.mult
            )

        def normalize(i, on_act=False):
            # TensorScalarPtr (per-partition scalar APs) is DVE-only; the
            # ACT variant uses Identity with per-partition scale/bias.
            if on_act:
                nc.scalar.activation(
                    out_sb[:, i, :], v_sb[:, i, :], AF.Identity,
                    bias=nmr_sb[:, i:i + 1], scale=rstd_sb[:, i:i + 1],
                )
            else:
                nc.vector.tensor_scalar(
                    out_sb[:, i, :], v_sb[:, i, :],
                    rstd_sb[:, i:i + 1], nmr_sb[:, i:i + 1], OP.mult, OP.add,
                )
            if need_gamma:
                nc.vector.tensor_mul(out_sb[:, i, :], out_sb[:, i, :], g_bc[:])
            if need_beta:
                nc.vector.tensor_add(out_sb[:, i, :], out_sb[:, i, :], be_bc[:])

        out_view = out_d.rearrange("(t p) d -> p t d", p=P)
        for i in range(NT):
            pab = ap_.tile([P, 512], f32)
            pa = pab[:, 0:D]
            for j in range(NT):
                nc.tensor.matmul(
                    pa,
                    adj_sb[:, j, i * P:(i + 1) * P],
                    yhi_sb[:, j, :],
                    start=(j == 0),
                    stop=(j == NT - 1),
                )
            if need_b:
                zsb = work.tile([P, D], f32, tag="zsb")
                nc.vector.tensor_tensor(zsb[:], pa, b_bc[:], OP.add)
                zin = zsb[:]
            else:
                zin = pa
            e = work.tile([P, D], bf16, tag="e")
            nc.scalar.activation(e[:], zin, AF.Exp)
            rx = work.tile([P, D], bf16, tag="rx")
            nc.vector.scalar_tensor_tensor(
                rx[:], zin, 0.0, xm1_sb[:, i, :], OP.max, OP.add
            )
            nc.vector.scalar_tensor_tensor(
                v_sb[:, i, :], e[:], 1.0, rx[:], OP.min, OP.add,
                accum_out=sum_sb[:, i:i + 1],
            )
            sq = work.tile([P, D], bf16, tag="sq")
            nc.scalar.activation(
                sq[:], v_sb[:, i, :], AF.Square, accum_out=sumsq_sb[:, i:i + 1]
            )
            if i == 3:
                stats_batch(0)
                normalize(0, on_act=True)
                normalize(1, on_act=True)
                normalize(2)
                normalize(3)
                nc.sync.dma_start(out=out_view[:, 0:4, :], in_=out_sb[:, 0:4, :])
            if i == 7:
                stats_batch(1)
                normalize(4)
                nc.sync.dma_start(out=out_view[:, 4, :], in_=out_sb[:, 4, :])
                normalize(5)
                nc.gpsimd.dma_start(out=out_view[:, 5, :], in_=out_sb[:, 5, :])
                normalize(6, on_act=True)
                nc.sync.dma_start(out=out_view[:, 6, :], in_=out_sb[:, 6, :])
                normalize(7, on_act=True)
                nc.gpsimd.dma_start(out=out_view[:, 7, :], in_=out_sb[:, 7, :])

        a_pool.__exit__(None, None, None)

    nc.compile()
    return nc


def _prepare_core_inputs(x_k, alphas, weights, W_T, b_proj, ln_gamma, ln_beta,
                         need_b, need_gamma, need_beta):
    import ml_dtypes

    bf = ml_dtypes.bfloat16
    xf = np.ascontiguousarray(x_k, dtype=np.float32)
    sq = np.sum(xf * xf, axis=-1, dtype=np.float32)
    xT = np.ascontiguousarray(xf.T)
    # Pre-blocked layouts: one contiguous DRAM row per SBUF partition.
    xhi = np.ascontiguousarray(
        xT.astype(bf).reshape(DC, P, N).transpose(1, 0, 2).reshape(P, DC * N)
    )
    xm1 = np.ascontiguousarray(
        (xf - np.float32(1.0)).astype(bf)
        .reshape(NT, P, D).transpose(1, 0, 2).reshape(P, NT * D)
    )
    nasq = np.stack([(-a) * sq for a in alphas]).astype(np.float32)
    nasq = np.ascontiguousarray(
        nasq.reshape(len(alphas), NT, P).transpose(2, 0, 1).reshape(P, -1)
    )
    m = {
        "xhi": xhi,
        "xm1": xm1,
        "whi": W_T.astype(bf),
        "nasq": nasq,
        "rbf": np.stack(
            [w * np.exp((-a) * sq.astype(np.float64)) for a, w in zip(alphas, weights)]
        ).astype(bf),
    }
    if need_b:
        m["brow"] = b_proj
    if need_gamma:
        m["grow"] = ln_gamma
    if need_beta:
        m["berow"] = ln_beta
    return m


def _specialize(inputs):
    x = np.asarray(inputs["x"], dtype=np.float32)
    log_sigmas = np.asarray(inputs["log_sigmas"], dtype=np.float32)
    W_proj = np.asarray(inputs["W_proj"], dtype=np.float32)
    b_proj = np.ascontiguousarray(np.asarray(inputs["b_proj"], dtype=np.float32))
    ln_gamma = np.ascontiguousarray(np.asarray(inputs["ln_gamma"], dtype=np.float32))
    ln_beta = np.ascontiguousarray(np.asarray(inputs["ln_beta"], dtype=np.float32))

    sigmas = np.exp(log_sigmas)
    denoms = (np.float32(2.0) * sigmas * sigmas + np.float32(1e-6)).astype(np.float32)
    uniq, counts = np.unique(denoms, return_counts=True)
    alphas = (np.float32(1.0) / uniq).astype(np.float32)
    weights = counts.astype(np.float32) / np.float32(H)
    two_alphas = tuple(float(2.0 * a) for a in alphas)

    need_b = bool(np.any(b_proj != 0))
    need_gamma = not bool(np.all(ln_gamma == 1))
    need_beta = bool(np.any(ln_beta != 0))
    return (x, W_proj, b_proj, ln_gamma, ln_beta, alphas, weights, two_alphas,
            need_b, need_gamma, need_beta)


def kernel(**inputs):
    from concourse import bass_utils

    (x, W_proj, b_proj, ln_gamma, ln_beta, alphas, weights, two_alphas,
     need_b, need_gamma, need_beta) = _specialize(inputs)

    assert x.shape == (B, N, D), x.shape

    key = (len(alphas), two_alphas, tuple(float(v) for v in weights),
           need_b, need_gamma, need_beta)
    if key not in _PROGRAM_CACHE:
        _PROGRAM_CACHE[key] = _build_program(
            len(alphas), two_alphas, need_b, need_gamma, need_beta
        )
    nc = _PROGRAM_CACHE[key]

    W_T = np.ascontiguousarray(W_proj.T)
    in_maps = [
        _prepare_core_inputs(x[k], alphas, weights, W_T, b_proj, ln_gamma, ln_beta,
                             need_b, need_gamma, need_beta)
        for k in range(B)
    ]

    res = bass_utils.run_bass_kernel_spmd(nc, in_maps, core_ids=list(range(B)))
    out = np.stack([np.asarray(res.results[k]["out"]) for k in range(B)])
    return out.astype(np.float32)


if __name__ == "__main__":
    import reference as R

    inp = R.setup_inputs()
    got = kernel(**{k: np.asarray(v) for k, v in inp.items()})
    print("out shape", got.shape, got.dtype)
